# revision 1
# baseline (speedup 1.0000x reference)
"""Trainium2 Bass kernel for nn_DecoderAttention (B=32, LQ=256, LK=2048, D=512, H=8).

Strategy: data-parallel over batch across 8 NeuronCores (4 batch items each).
Per batch item, everything is computed transposed-side so that all matmul
contractions sit on the partition dim with no runtime DMA transposes:

  kT  = PE-transpose(k)                      [D, LK]   (fp32r)
  kpT = Wk^T-chunks @ kT, k2T likewise       [D, LK]
  vp  = kT-chunks^T @ Wv^T (natural [LK, D]) + ones column per head (Z fold)
  qT, qpT                                    [D, LQ]
  per head: ST = kpT_h^T @ qpT_h             [LK, LQ]  (row-paired across the
            two heads sharing a 128-partition tile)
            E = exp(ST/8 + maskbias)  (ACT, fused mask, exact zeros)
            U = [vp_h | 1]^T @ E  -> rows 0:64 ctx, row 64 = Z
            ctx_h = U[0:64] / Z  (+bv)       [HD, LQ]
  ncT = per-head Wo_h^T @ ctx_h              [D, LQ]
  A   = ncT^T-chunks @ k2T; out = mask_fill(10*tanh(A/sqrt(D)))

LK is processed in 2 halves to fit SBUF; U/Z accumulate across halves via a
small SBUF staging tile. All matmuls run in fp32r (full PE rate at N>=256).
"""
import sys

sys.path.insert(0, "/opt/trn_rl_repo")

import numpy as np

import concourse.bass as bass
import concourse.bacc as bacc
import concourse.mybir as mybir
import concourse.tile as tile
from concourse import bass_utils
from concourse.masks import make_identity

F32 = mybir.dt.float32
F32R = mybir.dt.float32r
U8 = mybir.dt.uint8
AF = mybir.ActivationFunctionType

B, LQ, LK, D, H = 32, 256, 2048, 512, 8
HD = D // H              # 64
NCORES = 8
BPC = B // NCORES        # 4 batch items per core
NHALF = 2
LH = LK // NHALF         # 1024
NLB_H = LH // 128        # 8 l-blocks per half
CLIP = 10.0
FLOAT_MIN = -3.4e38
ISQ_HD = 0.125           # 1/sqrt(64)
ISQ_D = float(1.0 / np.sqrt(512.0))
MASK_BIG = -1e30
W_NAMES = ("Wq", "Wk", "Wv", "Wks")   # Wo handled per-head separately
B_OF_W = {"Wq": "bq", "Wk": "bk", "Wv": "bv", "Wo": "bo", "Wks": "bks"}

import os
MM_DT = {"f32r": F32R, "bf16": mybir.dt.bfloat16, "fp16": mybir.dt.float16}[os.environ.get("KDTYPE", "fp16")]

TRACE = False
LAST_RESULTS = None
_CACHE = {}


def _build(reps=1):
    nc = bacc.Bacc("TRN2", target_bir_lowering=False, debug=False)
    q_d = nc.dram_tensor("q", [BPC, LQ, D], F32, kind="ExternalInput").ap()
    k_d = nc.dram_tensor("k", [BPC, LK, D], F32, kind="ExternalInput").ap()
    m_d = nc.dram_tensor("mask", [BPC, LK], U8, kind="ExternalInput").ap()
    w_d = {n: nc.dram_tensor(n, [D, D], F32, kind="ExternalInput").ap()
           for n in W_NAMES + ("Wo",)}
    b_d = {n: nc.dram_tensor(B_OF_W[n], [D], F32, kind="ExternalInput").ap()
           for n in W_NAMES + ("Wo",)}
    out_d = nc.dram_tensor("out", [BPC, LQ, LK], F32, kind="ExternalOutput").ap()

    lowp = nc.allow_low_precision("bf16 matmul operands by design")
    lowp.__enter__()
    with tile.TileContext(nc) as tc:
        with (
            tc.tile_pool(name="c1", bufs=1) as c1,          # persistent consts
            tc.tile_pool(name="p1", bufs=1) as p1,          # per-batch big tiles
            tc.tile_pool(name="nat", bufs=5) as natp,       # [128,512] f32 staging
            tc.tile_pool(name="vpap", bufs=8) as vpap,
            tc.tile_pool(name="etp", bufs=4) as etp,
            tc.tile_pool(name="u0p", bufs=8) as u0p,
            tc.tile_pool(name="ctxp", bufs=8) as ctxp,
            tc.tile_pool(name="smal", bufs=2) as smal,      # small working tiles
            tc.tile_pool(name="mbp", bufs=16) as mbp,
            tc.tile_pool(name="thp", bufs=2) as thp,
            tc.tile_pool(name="mm_ps", bufs=3, space="PSUM") as mm_ps,
            tc.tile_pool(name="st_ps", bufs=3, space="PSUM") as st_ps,
            tc.tile_pool(name="u_ps", bufs=2, space="PSUM") as u_ps,
        ):
            # ---------------- one-time setup ----------------
            ident = c1.tile([128, 128], F32, tag="ident", name="ident")
            make_identity(nc, ident)
            ones_f = c1.tile([128, 1], F32, tag="ones_f", name="ones_f")
            nc.vector.memset(ones_f, 1.0)
            onr_f = c1.tile([128, 64], F32, tag="onr_f", name="onr_f")
            nc.vector.memset(onr_f, 1.0)
            onr = c1.tile([128, 64], MM_DT, tag="onr", name="onr")
            nc.vector.tensor_copy(onr, onr_f)
            fmin = c1.tile([128, 512], F32, tag="fmin", name="fmin")
            nc.vector.memset(fmin, FLOAT_MIN)

            # biases as [128,1] column chunks
            bcol = {}
            for wn in W_NAMES + ("Wo",):
                bn = B_OF_W[wn]
                ap3 = b_d[wn].rearrange("(c p one) -> c p one", p=128, one=1)
                for c in range(4):
                    t = c1.tile([128, 1], F32, tag=f"b_{bn}_{c}", name=f"b_{bn}_{c}")
                    nc.sync.dma_start(out=t, in_=ap3[c])
                    bcol[(bn, c)] = t
            # bv per head [64,1]
            bvh = []
            aph = b_d["Wv"].rearrange("(h p one) -> h p one", p=64, one=1)
            for h in range(H):
                t = c1.tile([64, 1], F32, tag=f"bvh{h}", name=f"bvh{h}")
                nc.sync.dma_start(out=t, in_=aph[h])
                bvh.append(t)

            # transposed weights WT[w][dk] = W^T chunk [128(d), 512(n)] fp32r
            WT = {}
            for wn in W_NAMES:
                wnat = []
                for nj in range(4):
                    t = natp.tile([128, 512], F32, tag="nat", name="nat")
                    nc.sync.dma_start(out=t, in_=w_d[wn][nj * 128:(nj + 1) * 128, :])
                    wnat.append(t)
                for dk in range(4):
                    pt = mm_ps.tile([128, 512], F32, tag="mm", name="mm")
                    for nj in range(4):
                        nc.tensor.transpose(
                            pt[:, nj * 128:(nj + 1) * 128],
                            wnat[nj][:, dk * 128:(dk + 1) * 128], ident)
                    wt = c1.tile([128, 512], MM_DT, tag=f"wt_{wn}_{dk}", name=f"wt_{wn}_{dk}")
                    nc.vector.tensor_copy(wt, pt)
                    WT[(wn, dk)] = wt
            # Wo per head: WoT_h [64(e in head), 512(n)] fp32r
            WoTh = []
            wnat = []
            for nj in range(4):
                t = natp.tile([128, 512], F32, tag="nat", name="nat")
                nc.sync.dma_start(out=t, in_=w_d["Wo"][nj * 128:(nj + 1) * 128, :])
                wnat.append(t)
            for h in range(H):
                pt = mm_ps.tile([128, 512], F32, tag="mm", name="mm")
                for nj in range(4):
                    nc.tensor.transpose(
                        pt[0:64, nj * 128:(nj + 1) * 128],
                        wnat[nj][:, h * 64:(h + 1) * 64], ident)
                wt = c1.tile([64, 512], MM_DT, tag=f"woth{h}", name=f"woth{h}")
                nc.vector.tensor_copy(wt, pt[0:64, :])
                WoTh.append(wt)

            # ---------------- per batch item ----------------
            for bi in [b for _ in range(reps) for b in range(BPC)]:
                # mask: broadcast row to all partitions (uint8)
                m_row = m_d[bi]
                bcast = bass.AP(tensor=m_row.tensor, offset=m_row.offset,
                                ap=[[0, 128]] + m_row.ap)
                masku8 = p1.tile([128, LK], U8, tag="masku8", name="masku8")
                nc.gpsimd.dma_start(out=masku8, in_=bcast)
                # maskbias tiles [128,1] per l-chunk: transpose f32 copy of mask
                mb = []
                for g in range(4):
                    pt = mm_ps.tile([128, 512], F32, tag="mm", name="mm")
                    for c in range(4):
                        lcg = g * 4 + c
                        mt = smal.tile([128, 128], F32, tag="mtmp", name="mtmp")
                        nc.vector.tensor_copy(
                            mt, masku8[:, lcg * 128:(lcg + 1) * 128])
                        nc.tensor.transpose(
                            pt[:, c * 128:(c + 1) * 128], mt, ident)
                    for c in range(4):
                        t = mbp.tile([128, 1], F32, tag="mb", name="mb")
                        nc.scalar.mul(t, pt[:, c * 128:c * 128 + 1], MASK_BIG)
                        mb.append(t)

                # q transposed + projected
                qnat = []
                for mi in range(2):
                    t = natp.tile([128, 512], F32, tag="nat", name="nat")
                    nc.sync.dma_start(
                        out=t, in_=q_d[bi, mi * 128:(mi + 1) * 128, :])
                    qnat.append(t)
                qTr = []
                for dk in range(4):
                    pt = mm_ps.tile([128, 512], F32, tag="mm", name="mm")
                    for mi in range(2):
                        nc.tensor.transpose(
                            pt[:, mi * 128:(mi + 1) * 128],
                            qnat[mi][:, dk * 128:(dk + 1) * 128], ident)
                    t = p1.tile([128, 256], MM_DT, tag=f"qtr{dk}", name=f"qtr{dk}")
                    nc.vector.tensor_copy(t, pt[:, 0:256])
                    qTr.append(t)
                qpTr = []
                for nj in range(4):
                    pt = mm_ps.tile([128, 512], F32, tag="mm", name="mm")
                    for dk in range(4):
                        nc.tensor.matmul(
                            pt[:, 0:256], WT[("Wq", dk)][:, nj * 128:(nj + 1) * 128],
                            qTr[dk], start=(dk == 0), stop=(dk == 3))
                    t = p1.tile([128, 256], MM_DT, tag=f"qptr{nj}", name=f"qptr{nj}")
                    nc.scalar.activation(t, pt[:, 0:256], AF.Identity,
                                         bias=bcol[("bq", nj)][:, :], scale=1.0)
                    qpTr.append(t)

                # k2T spans both halves
                k2Tr = [p1.tile([128, LK], MM_DT, tag=f"k2t{nj}", name=f"k2t{nj}") for nj in range(4)]

                u0 = {}          # (pair, head-in-pair) -> [65,256] f32 staging
                ctxh = []        # per-head ctx tiles [64,256] f32r

                for hf in range(NHALF):
                    lbase = hf * LH
                    # load k blocks for this half
                    kTr = [p1.tile([128, LH], MM_DT, tag=f"kt{dk}", name=f"kt{dk}")
                           for dk in range(4)]
                    for g in range(2):          # groups of 4 l-blocks
                        knat = []
                        for li in range(4):
                            t = natp.tile([128, 512], F32, tag="nat", name="nat")
                            lb = lbase + (g * 4 + li) * 128
                            nc.sync.dma_start(out=t, in_=k_d[bi, lb:lb + 128, :])
                            knat.append(t)
                        for dk in range(4):
                            pt = mm_ps.tile([128, 512], F32, tag="mm", name="mm")
                            for li in range(4):
                                nc.tensor.transpose(
                                    pt[:, li * 128:(li + 1) * 128],
                                    knat[li][:, dk * 128:(dk + 1) * 128], ident)
                            nc.vector.tensor_copy(
                                kTr[dk][:, g * 512:(g + 1) * 512], pt)

                    # projections for this half
                    kpTr = [p1.tile([128, LH], MM_DT, tag=f"kpt{t_}", name=f"kpt{t_}")
                            for t_ in range(4)]
                    for nj in range(4):
                        for g in range(2):
                            pt = mm_ps.tile([128, 512], F32, tag="mm", name="mm")
                            for dk in range(4):
                                nc.tensor.matmul(
                                    pt, WT[("Wk", dk)][:, nj * 128:(nj + 1) * 128],
                                    kTr[dk][:, g * 512:(g + 1) * 512],
                                    start=(dk == 0), stop=(dk == 3))
                            nc.vector.tensor_scalar_add(
                                out=kpTr[nj][:, g * 512:(g + 1) * 512],
                                in0=pt, scalar1=bcol[("bk", nj)][:, :])
                    for nj in range(4):
                        for g in range(2):
                            pt = mm_ps.tile([128, 512], F32, tag="mm", name="mm")
                            for dk in range(4):
                                nc.tensor.matmul(
                                    pt, WT[("Wks", dk)][:, nj * 128:(nj + 1) * 128],
                                    kTr[dk][:, g * 512:(g + 1) * 512],
                                    start=(dk == 0), stop=(dk == 3))
                            nc.scalar.activation(
                                k2Tr[nj][:, lbase + g * 512:lbase + (g + 1) * 512],
                                pt, AF.Identity,
                                bias=bcol[("bks", nj)][:, :], scale=1.0)
                    vpa = []
                    for lb in range(NLB_H):
                        pt = mm_ps.tile([128, 512], F32, tag="mm", name="mm")
                        for dk in range(4):
                            nc.tensor.matmul(
                                pt, kTr[dk][:, lb * 128:(lb + 1) * 128],
                                WT[("Wv", dk)], start=(dk == 0), stop=(dk == 3))
                        vt = vpap.tile([128, H * 65], MM_DT, tag="vpa", name="vpa")
                        vt3 = vt.rearrange("p (h c) -> p h c", h=H)
                        nc.vector.tensor_copy(
                            vt3[:, :, 0:64],
                            pt.rearrange("p (h c) -> p h c", h=H))
                        src = ones_f.to_broadcast([128, H, 1])
                        nc.vector.tensor_copy(vt3[:, :, 64:65], src)
                        vpa.append(vt)

                    # attention per head pair
                    for t_ in range(4):
                        ups = [u_ps.tile([65, 256], F32, tag="u", name="u") for _ in range(2)]
                        for lc in range(NLB_H):
                            lcg = hf * NLB_H + lc
                            stp = []
                            for hh in range(2):
                                sp = st_ps.tile([128, 256], F32, tag="st", name="st")
                                nc.tensor.matmul(
                                    sp,
                                    kpTr[t_][hh * 64:(hh + 1) * 64,
                                             lc * 128:(lc + 1) * 128],
                                    qpTr[t_][hh * 64:(hh + 1) * 64, :],
                                    start=True, stop=True)
                                stp.append(sp)
                            for hh in range(2):
                                et = etp.tile([128, 256], MM_DT, tag="et", name="et")
                                nc.scalar.activation(
                                    et, stp[hh], AF.Exp,
                                    bias=mb[lcg][:, :], scale=ISQ_HD)
                                h = 2 * t_ + hh
                                nc.tensor.matmul(
                                    ups[hh],
                                    vpa[lc][:, h * 65:(h + 1) * 65],
                                    et, start=(lc == 0), stop=(lc == NLB_H - 1),
                                    skip_group_check=True)
                        if hf == 0:
                            for hh in range(2):
                                t = u0p.tile([65, 256], F32, tag="u0", name="u0")
                                nc.vector.tensor_copy(t, ups[hh])
                                u0[(t_, hh)] = t
                        else:
                            for hh in range(2):
                                h = 2 * t_ + hh
                                tmp = smal.tile([65, 256], F32, tag="tmp", name="tmp")
                                nc.vector.tensor_add(tmp, u0[(t_, hh)], ups[hh])
                                tmpr = smal.tile([65, 256], MM_DT, tag="tmpr", name="tmpr")
                                nc.vector.tensor_copy(
                                    tmpr[64:65, :], tmp[64:65, :])
                                zb = mm_ps.tile([64, 256], F32, tag="mm", name="mm")
                                nc.tensor.matmul(
                                    zb, onr[64:65, :], tmpr[64:65, :],
                                    start=True, stop=True)
                                rz = smal.tile([64, 256], F32, tag="rz", name="rz")
                                nc.vector.reciprocal(rz, zb)
                                ct = ctxp.tile([64, 256], MM_DT, tag="ctx", name="ctx")
                                nc.vector.tensor_mul(ct, tmp[0:64, :], rz)
                                nc.scalar.activation(
                                    ct, ct, AF.Identity,
                                    bias=bvh[h][:, :], scale=1.0)
                                ctxh.append(ct)

                # out_proj (transposed): ncT[n, m]
                ncTr = []
                for nj in range(4):
                    pt = mm_ps.tile([128, 512], F32, tag="mm", name="mm")
                    for h in range(H):
                        nc.tensor.matmul(
                            pt[:, 0:256], WoTh[h][:, nj * 128:(nj + 1) * 128],
                            ctxh[h], start=(h == 0), stop=(h == H - 1))
                    t = p1.tile([128, 256], MM_DT, tag=f"nct{nj}", name=f"nct{nj}")
                    nc.scalar.activation(t, pt[:, 0:256], AF.Identity,
                                         bias=bcol[("bo", nj)][:, :], scale=1.0)
                    ncTr.append(t)

                # final scores + tanh clip + mask fill
                for mi in range(2):
                    for lg in range(4):
                        pt = mm_ps.tile([128, 512], F32, tag="mm", name="mm")
                        for nk in range(4):
                            nc.tensor.matmul(
                                pt, ncTr[nk][:, mi * 128:(mi + 1) * 128],
                                k2Tr[nk][:, lg * 512:(lg + 1) * 512],
                                start=(nk == 0), stop=(nk == 3))
                        th = thp.tile([128, 512], F32, tag="th", name="th")
                        nc.scalar.activation(th, pt, AF.Tanh,
                                             bias=0.0, scale=ISQ_D)
                        nc.gpsimd.tensor_scalar_mul(th, th, CLIP)
                        nc.vector.copy_predicated(
                            th, masku8[:, lg * 512:(lg + 1) * 512], fmin)
                        nc.sync.dma_start(
                            out=out_d[bi, mi * 128:(mi + 1) * 128,
                                      lg * 512:(lg + 1) * 512],
                            in_=th)
    lowp.__exit__(None, None, None)
    nc.finalize()
    return nc


def kernel(**inputs):
    global LAST_RESULTS
    import os
    reps = int(os.environ.get("KERNEL_REPS", "1"))
    key = ("nc", reps)
    if key not in _CACHE:
        _CACHE[key] = _build(reps)
    nc = _CACHE[key]

    q = np.ascontiguousarray(np.asarray(inputs["q"], dtype=np.float32))
    k = np.ascontiguousarray(np.asarray(inputs["k"], dtype=np.float32))
    mask = np.ascontiguousarray(np.asarray(inputs["mask"]).astype(np.uint8))
    ws = {n: np.ascontiguousarray(np.asarray(inputs[n], dtype=np.float32))
          for n in W_NAMES + ("Wo",)}
    bs = {B_OF_W[n]: np.ascontiguousarray(
        np.asarray(inputs[B_OF_W[n]], dtype=np.float32))
        for n in W_NAMES + ("Wo",)}

    in_maps = []
    for ci in range(NCORES):
        sl = slice(ci * BPC, (ci + 1) * BPC)
        im = {"q": q[sl], "k": k[sl], "mask": mask[sl]}
        im.update(ws)
        im.update(bs)
        in_maps.append(im)

    res = bass_utils.run_bass_kernel_spmd(
        nc, in_maps, core_ids=list(range(NCORES)), trace=TRACE)
    LAST_RESULTS = res
    out = np.concatenate([res.results[ci]["out"] for ci in range(NCORES)], axis=0)
    return out



# revision 19
# speedup vs baseline: 1.4844x; 1.4844x over previous
"""Trainium2 Bass kernel for nn_DecoderAttention (B=32, LQ=256, LK=2048, D=512, H=8).

Data-parallel over batch across 8 NeuronCores (4 batch items each).
All matmuls run in bf16 (1 col/cycle at warm 2.4GHz PE clock).

Per batch item (transposed-side layout, contraction always on partitions):
  k loaded via gpsimd cast-DMA (f32 DRAM -> bf16 SBUF), PE-transposed to
  kT[d, l] in l-groups of 512; kp/k2/vp projections stream per group.
  Per head-pair t_: S^T[l, q] for both heads into one [128,512] PSUM bank
  (cols 0:256 / 256:512) -> ONE Exp activation (mask bias per l-partition)
  -> E bf16; U = [v | 1]^T E accumulates over all 16 l-blocks in one PSUM
  bank (even head rows 0:65 cols 0:256 with Z last; odd head rows 63:128
  cols 256:512 with Z first, so ctx rows land lane-aligned at 0:64/64:128).
  Z broadcast via ones-matmul, one reciprocal, two lane-aligned muls,
  + bv as per-partition bias -> ctx pair [128, 256] bf16.
  out_proj = 4-step accumulation over head-pairs; final scores = ncT^T@k2T,
  tanh*CLIP (ACT tanh + DVE mul), mask fill via copy_predicated, DMA out.

Cross-batch overlap via bufs=2 tile pools keeps the PE HAM-warm.
"""
import sys

sys.path.insert(0, "/opt/trn_rl_repo")

import numpy as np

import concourse.bass as bass
import concourse.bacc as bacc
import concourse.mybir as mybir
import concourse.tile as tile
from concourse import bass_utils
from concourse.masks import make_identity

F32 = mybir.dt.float32
BF16 = mybir.dt.bfloat16
U8 = mybir.dt.uint8
AF = mybir.ActivationFunctionType

B, LQ, LK, D, H = 32, 256, 2048, 512, 8
HD = D // H              # 64
NCORES = 8
BPC = B // NCORES        # 4 batch items per core
NLB = LK // 128          # 16 l-blocks
NG = LK // 512           # 4 l-groups
CLIP = 10.0
FLOAT_MIN = -3.4e38
ISQ_HD = 0.125           # 1/sqrt(64)
ISQ_D = float(1.0 / np.sqrt(512.0))
MASK_BIG = -1e30
W_NAMES = ("Wq", "Wk", "Wv", "Wks", "Wo")
B_OF_W = {"Wq": "bq", "Wk": "bk", "Wv": "bv", "Wo": "bo", "Wks": "bks"}

TRACE = False
LAST_RESULTS = None
_CACHE = {}


def _build(reps=1):
    nc = bacc.Bacc("TRN2", target_bir_lowering=False, debug=False)
    q_d = nc.dram_tensor("q", [BPC, LQ, D], F32, kind="ExternalInput").ap()
    k_d = nc.dram_tensor("k", [BPC, LK, D], F32, kind="ExternalInput").ap()
    m_d = nc.dram_tensor("mask", [BPC, LK], U8, kind="ExternalInput").ap()
    w_d = {n: nc.dram_tensor(n, [D, D], F32, kind="ExternalInput").ap()
           for n in W_NAMES}
    b_d = {n: nc.dram_tensor(B_OF_W[n], [D], F32, kind="ExternalInput").ap()
           for n in W_NAMES}
    out_d = nc.dram_tensor("out", [BPC, LQ, LK], F32, kind="ExternalOutput").ap()

    lowp = nc.allow_low_precision("bf16 matmul operands by design")
    lowp.__enter__()
    with tile.TileContext(nc) as tc:
        with (
            tc.tile_pool(name="c1", bufs=1) as c1,          # persistent consts
            tc.tile_pool(name="pb", bufs=2) as pb,          # per-batch persistents
            tc.tile_pool(name="vpap", bufs=24) as vpap,
            tc.tile_pool(name="knp", bufs=5) as knp,        # k natural staging
            tc.tile_pool(name="ktp", bufs=8) as ktp,        # kT group tiles
            tc.tile_pool(name="etp", bufs=3) as etp,        # exp output tiles
            tc.tile_pool(name="thp", bufs=2) as thp,        # final output staging
            tc.tile_pool(name="smal", bufs=2) as smal,      # small working tiles
            tc.tile_pool(name="tr_ps", bufs=2, space="PSUM") as tr_ps,
            tc.tile_pool(name="mm_ps", bufs=2, space="PSUM") as mm_ps,
            tc.tile_pool(name="st_ps", bufs=2, space="PSUM") as st_ps,
            tc.tile_pool(name="u_ps", bufs=2, space="PSUM") as u_ps,
        ):
            # ---------------- one-time setup ----------------
            identf = knp.tile([128, 128], F32, tag="identf", name="identf")
            make_identity(nc, identf)
            ident = c1.tile([128, 128], BF16, tag="ident", name="ident")
            nc.vector.tensor_copy(ident, identf)
            onr = c1.tile([128, 64], BF16, tag="onr", name="onr")
            nc.vector.memset(onr, 1.0)
            fmin = c1.tile([128, 1], F32, tag="fmin", name="fmin")
            nc.vector.memset(fmin, FLOAT_MIN)

            # biases as [128,1] column chunks (f32)
            bcol = {}
            for wn in W_NAMES:
                bn = B_OF_W[wn]
                ap3 = b_d[wn].rearrange("(c p one) -> c p one", p=128, one=1)
                for c in range(4):
                    t = c1.tile([128, 1], F32, tag=f"b_{bn}_{c}", name=f"b_{bn}_{c}")
                    nc.sync.dma_start(out=t, in_=ap3[c])
                    bcol[(bn, c)] = t
            # bv per head [64,1]
            bvh = []
            aph = b_d["Wv"].rearrange("(h p one) -> h p one", p=64, one=1)
            for h in range(H):
                t = c1.tile([64, 1], F32, tag=f"bvh{h}", name=f"bvh{h}")
                nc.sync.dma_start(out=t, in_=aph[h])
                bvh.append(t)

            # transposed weights WT[(wn, dk)] = [128(din chunk), 512(dout)] bf16
            WT = {}
            for wn in ("Wq", "Wk", "Wv", "Wks"):
                wnat = []
                for nj in range(4):
                    t = knp.tile([128, 512], BF16, tag="knat", name="knat")
                    nc.gpsimd.dma_start(
                        out=t, in_=w_d[wn][nj * 128:(nj + 1) * 128, :])
                    wnat.append(t)
                for dk in range(4):
                    pt = tr_ps.tile([128, 512], BF16, tag="tr", name="tr")
                    for nj in range(4):
                        nc.tensor.transpose(
                            pt[:, nj * 128:(nj + 1) * 128],
                            wnat[nj][:, dk * 128:(dk + 1) * 128], ident)
                    wt = c1.tile([128, 512], BF16, tag=f"wt_{wn}_{dk}",
                                 name=f"wt_{wn}_{dk}")
                    nc.any.tensor_copy(wt, pt)
                    WT[(wn, dk)] = wt
            # Wo per head: WoTh[h] = [64(din in head), 512(dout)] bf16, base 0
            WoTh = []
            wnat = []
            for nj in range(4):
                t = knp.tile([128, 512], BF16, tag="knat", name="knat")
                nc.gpsimd.dma_start(
                    out=t, in_=w_d["Wo"][nj * 128:(nj + 1) * 128, :])
                wnat.append(t)
            for h in range(H):
                pt = tr_ps.tile([128, 512], BF16, tag="tr", name="tr")
                for nj in range(4):
                    nc.tensor.transpose(
                        pt[0:64, nj * 128:(nj + 1) * 128],
                        wnat[nj][:, h * 64:(h + 1) * 64], ident)
                wt = c1.tile([64, 512], BF16, tag=f"woth{h}", name=f"woth{h}")
                nc.any.tensor_copy(wt, pt[0:64, :])
                WoTh.append(wt)

            # ---------------- per batch item ----------------
            for bi in [b for _ in range(reps) for b in range(BPC)]:
                # mask: broadcast row to all partitions (uint8) for final fill
                m_row = m_d[bi]
                bcast = bass.AP(tensor=m_row.tensor, offset=m_row.offset,
                                ap=[[0, 128]] + m_row.ap)
                masku8 = pb.tile([128, LK], U8, tag="masku8", name="masku8")
                nc.gpsimd.dma_start(out=masku8, in_=bcast)
                # mask bias columns mb[:, lcg] = -1e30 * mask[lcg*128 + p]
                m16 = smal.tile([16, 128], U8, tag="m16", name="m16")
                nc.sync.dma_start(
                    out=m16, in_=m_row.rearrange("(c p) -> c p", c=16))
                m16f = smal.tile([16, 128], BF16, tag="m16f", name="m16f")
                nc.vector.tensor_copy(m16f, m16)
                mpt = tr_ps.tile([128, 512], BF16, tag="tr", name="tr")
                nc.tensor.transpose(mpt[:, 0:16], m16f, ident[0:16, 0:16])
                mb = pb.tile([128, 16], F32, tag="mb", name="mb")
                nc.vector.tensor_scalar_mul(mb, mpt[:, 0:16], MASK_BIG)

                # q: cast-load + transpose + project
                qnat = []
                for mi in range(2):
                    t = knp.tile([128, 512], BF16, tag="knat", name="knat")
                    nc.gpsimd.dma_start(
                        out=t, in_=q_d[bi, mi * 128:(mi + 1) * 128, :])
                    qnat.append(t)
                qTr = []
                for dk in range(4):
                    pt = tr_ps.tile([128, 512], BF16, tag="tr", name="tr")
                    for mi in range(2):
                        nc.tensor.transpose(
                            pt[:, mi * 128:(mi + 1) * 128],
                            qnat[mi][:, dk * 128:(dk + 1) * 128], ident)
                    t = c1.tile([128, 256], BF16, tag=f"qtr{dk}", name=f"qtr{dk}")
                    nc.any.tensor_copy(t, pt[:, 0:256])
                    qTr.append(t)
                qpTr = []
                for nj in range(4):
                    pt = mm_ps.tile([128, 512], F32, tag="mm", name="mm")
                    for dk in range(4):
                        nc.tensor.matmul(
                            pt[:, 0:256], WT[("Wq", dk)][:, nj * 128:(nj + 1) * 128],
                            qTr[dk], start=(dk == 0), stop=(dk == 3))
                    t = pb.tile([128, 256], BF16, tag=f"qptr{nj}", name=f"qptr{nj}")
                    nc.scalar.activation(t, pt[:, 0:256], AF.Identity,
                                         bias=bcol[("bq", nj)][:, :], scale=1.0)
                    qpTr.append(t)
                qpOd = []
                for nj in range(4):
                    t = pb.tile([64, 256], BF16, tag=f"qpo{nj}", name=f"qpo{nj}")
                    nc.sync.dma_start(out=t, in_=qpTr[nj][64:128, :])
                    qpOd.append(t)

                kpTr = [pb.tile([128, LK], BF16, tag=f"kpt{nj}", name=f"kpt{nj}")
                        for nj in range(4)]
                # odd-head rows moved to base partition 0 (lane shift via DMA)
                # so every attention matmul operand sits at base 0.
                kpOd = [pb.tile([64, LK], BF16, tag=f"kpo{nj}", name=f"kpo{nj}")
                        for nj in range(4)]
                k2Tr = [pb.tile([128, LK], BF16, tag=f"k2t{nj}", name=f"k2t{nj}")
                        for nj in range(4)]
                vpa = []

                for g in range(NG):
                    lbase = g * 512
                    knat = []
                    for li in range(4):
                        t = knp.tile([128, 512], BF16, tag="knat", name="knat")
                        nc.gpsimd.dma_start(
                            out=t, in_=k_d[bi, lbase + li * 128:lbase + (li + 1) * 128, :])
                        knat.append(t)
                    kTg = []
                    for dk in range(4):
                        pt = tr_ps.tile([128, 512], BF16, tag="tr", name="tr")
                        for li in range(4):
                            nc.tensor.transpose(
                                pt[:, li * 128:(li + 1) * 128],
                                knat[li][:, dk * 128:(dk + 1) * 128], ident)
                        t = ktp.tile([128, 512], BF16, tag="ktg", name="ktg")
                        nc.any.tensor_copy(t, pt)
                        kTg.append(t)
                    # kp projection (bias via DVE), k2 (bias via ACT)
                    for nj in range(4):
                        pt = mm_ps.tile([128, 512], F32, tag="mm", name="mm")
                        for dk in range(4):
                            nc.tensor.matmul(
                                pt, WT[("Wk", dk)][:, nj * 128:(nj + 1) * 128],
                                kTg[dk], start=(dk == 0), stop=(dk == 3))
                        nc.vector.tensor_scalar_add(
                            out=kpTr[nj][:, lbase:lbase + 512],
                            in0=pt, scalar1=bcol[("bk", nj)][:, :])
                        nc.sync.dma_start(
                            out=kpOd[nj][:, lbase:lbase + 512],
                            in_=kpTr[nj][64:128, lbase:lbase + 512])
                    for nj in range(4):
                        pt = mm_ps.tile([128, 512], F32, tag="mm", name="mm")
                        for dk in range(4):
                            nc.tensor.matmul(
                                pt, WT[("Wks", dk)][:, nj * 128:(nj + 1) * 128],
                                kTg[dk], start=(dk == 0), stop=(dk == 3))
                        nc.scalar.activation(
                            k2Tr[nj][:, lbase:lbase + 512], pt, AF.Identity,
                            bias=bcol[("bks", nj)][:, :], scale=1.0)
                    # vp natural [l, dout] -> vpa per head: [v(64) | one]
                    for lb in range(4):
                        pt = mm_ps.tile([128, 512], F32, tag="mm", name="mm")
                        for dk in range(4):
                            nc.tensor.matmul(
                                pt, kTg[dk][:, lb * 128:(lb + 1) * 128],
                                WT[("Wv", dk)], start=(dk == 0), stop=(dk == 3))
                        vt = vpap.tile([128, H * 65], BF16, tag="vpa", name="vpa")
                        vt3 = vt.rearrange("p (h c) -> p h c", c=65)
                        nc.vector.tensor_copy(
                            vt3[:, :, 0:64],
                            pt.rearrange("p (h c) -> p h c", c=64))
                        nc.vector.memset(vt3[:, :, 64:65], 1.0)
                        vpa.append(vt)

                # attention per head pair
                ctxh = []
                for t_ in range(4):
                    u = u_ps.tile([128, 512], F32, tag="u", name="u")
                    for lc in range(NLB):
                        sp = st_ps.tile([128, 512], F32, tag="st", name="st")
                        # hh=1 start=False: start clears the whole bank
                        nc.tensor.matmul(
                            sp[:, 0:256],
                            kpTr[t_][0:64, lc * 128:(lc + 1) * 128],
                            qpTr[t_][0:64, :],
                            start=True, stop=True, skip_group_check=True)
                        nc.tensor.matmul(
                            sp[:, 256:512],
                            kpOd[t_][:, lc * 128:(lc + 1) * 128],
                            qpOd[t_],
                            start=False, stop=True, skip_group_check=True)
                        et = etp.tile([128, 512], BF16, tag="et", name="et")
                        nc.scalar.activation(
                            et, sp, AF.Exp,
                            bias=mb[:, lc:lc + 1], scale=ISQ_HD)
                        nc.tensor.matmul(
                            u[0:65, 0:256],
                            vpa[lc][:, (2 * t_) * 65:(2 * t_) * 65 + 65],
                            et[:, 0:256], start=(lc == 0), stop=(lc == NLB - 1),
                            skip_group_check=True)
                        # start only on the bank's first matmul: start=True
                        # clears the WHOLE bank; the odd head's first matmul
                        # relies on has_written=0 -> overwrite semantics.
                        nc.tensor.matmul(
                            u[0:65, 256:512],
                            vpa[lc][:, (2 * t_ + 1) * 65:(2 * t_ + 1) * 65 + 65],
                            et[:, 256:512], start=False, stop=(lc == NLB - 1),
                            skip_group_check=True)
                    # Z row -> bf16, broadcast via ones-matmul, divide, + bv
                    zr = smal.tile([128, 512], BF16, tag="zr", name="zr")
                    nc.vector.tensor_copy(zr[64:65, :], u[64:65, :])
                    zb = mm_ps.tile([128, 512], F32, tag="mm", name="mm")
                    nc.tensor.matmul(zb[0:64, :], onr[64:65, :],
                                     zr[64:65, :], start=True, stop=True)
                    rz = smal.tile([64, 512], F32, tag="rz", name="rz")
                    nc.vector.reciprocal(rz, zb[0:64, :])
                    ct = c1.tile([64, 512], BF16, tag=f"ctx{t_}", name=f"ctx{t_}")
                    nc.vector.tensor_mul(ct, u[0:64, :], rz)
                    nc.vector.tensor_scalar_add(
                        out=ct[:, 0:256], in0=ct[:, 0:256],
                        scalar1=bvh[2 * t_][:, :])
                    nc.vector.tensor_scalar_add(
                        out=ct[:, 256:512], in0=ct[:, 256:512],
                        scalar1=bvh[2 * t_ + 1][:, :])
                    ctxh.append(ct)

                # out_proj: ncT[dout, q] = sum over heads (p=64 each)
                ncTr = []
                for nj in range(4):
                    pt = mm_ps.tile([128, 512], F32, tag="mm", name="mm")
                    for t_ in range(4):
                        for hh in range(2):
                            nc.tensor.matmul(
                                pt[:, 0:256],
                                WoTh[2 * t_ + hh][:, nj * 128:(nj + 1) * 128],
                                ctxh[t_][:, hh * 256:(hh + 1) * 256],
                                start=(t_ == 0 and hh == 0),
                                stop=(t_ == 3 and hh == 1))
                    t = c1.tile([128, 256], BF16, tag=f"nct{nj}", name=f"nct{nj}")
                    nc.scalar.activation(t, pt[:, 0:256], AF.Identity,
                                         bias=bcol[("bo", nj)][:, :], scale=1.0)
                    ncTr.append(t)

                # final scores + tanh clip + mask fill
                for mi in range(2):
                    for lg in range(4):
                        pt = mm_ps.tile([128, 512], F32, tag="mm", name="mm")
                        for nk in range(4):
                            nc.tensor.matmul(
                                pt, ncTr[nk][:, mi * 128:(mi + 1) * 128],
                                k2Tr[nk][:, lg * 512:(lg + 1) * 512],
                                start=(nk == 0), stop=(nk == 3))
                        th = thp.tile([128, 512], F32, tag="th", name="th")
                        nc.scalar.activation(th, pt, AF.Tanh,
                                             bias=0.0, scale=ISQ_D)
                        nc.vector.tensor_scalar_mul(th, th, CLIP)
                        nc.vector.copy_predicated(
                            th, masku8[:, lg * 512:(lg + 1) * 512],
                            fmin.to_broadcast([128, 512]))
                        nc.sync.dma_start(
                            out=out_d[bi, mi * 128:(mi + 1) * 128,
                                      lg * 512:(lg + 1) * 512],
                            in_=th)
    lowp.__exit__(None, None, None)
    nc.finalize()
    return nc


def kernel(**inputs):
    global LAST_RESULTS
    import os
    reps = int(os.environ.get("KERNEL_REPS", "1"))
    key = ("nc", reps)
    if key not in _CACHE:
        _CACHE[key] = _build(reps)
    nc = _CACHE[key]

    q = np.ascontiguousarray(np.asarray(inputs["q"], dtype=np.float32))
    k = np.ascontiguousarray(np.asarray(inputs["k"], dtype=np.float32))
    mask = np.ascontiguousarray(np.asarray(inputs["mask"]).astype(np.uint8))
    ws = {n: np.ascontiguousarray(np.asarray(inputs[n], dtype=np.float32))
          for n in W_NAMES}
    bs = {B_OF_W[n]: np.ascontiguousarray(
        np.asarray(inputs[B_OF_W[n]], dtype=np.float32))
        for n in W_NAMES}

    in_maps = []
    for ci in range(NCORES):
        sl = slice(ci * BPC, (ci + 1) * BPC)
        im = {"q": q[sl], "k": k[sl], "mask": mask[sl]}
        im.update(ws)
        im.update(bs)
        in_maps.append(im)

    res = bass_utils.run_bass_kernel_spmd(
        nc, in_maps, core_ids=list(range(NCORES)), trace=TRACE)
    LAST_RESULTS = res
    out = np.concatenate([res.results[ci]["out"] for ci in range(NCORES)], axis=0)
    return out
